# revision 1
# baseline (speedup 1.0000x reference)
"""DagEncoder (MLP + segment_sum) Trainium2 kernel, 8-core SPMD.

Contract: kernel(**inputs) takes the FULL unsharded inputs of
reference.setup_inputs() and returns the FULL [M, E] output.

Strategy (pure data parallelism over DAG segments):
  - 20000 segments split into 8 cores x 2500 segments; each core gets its
    node range. Within a core, segments are split into 2 "streams" so two
    nodes are processed per PE column (feature-major layout, 2x40 features
    stacked on partitions 0..79).
  - Host pads every segment to a multiple of 16 nodes (zero pad) and
    reorders nodes so that a 16:1 block reduction is expressible as 4
    contiguous half-adds (fold tree) per 1024-column chunk.
  - Device per chunk: mm1 (W1 blockdiag) -> relu evac (DVE) -> mm2
    (W2 blockdiag) -> relu evac (ACT) -> fold tree (GPSIMD) producing
    per-16-node-block partial sums of relu(h2).
  - Blocks are grouped into tiles of 128; each tile's partials are PE
    transposed to node-major and multiplied by a one-hot slot matrix S
    (block -> one of 32 output slots per tile), accumulating exact fp32
    segment sums in PSUM. A final W3 matmul maps 64->32 features.
  - Host scatters virtual slots back to global segments and applies the
    (counts * b3) term plus a pad-node bias correction (both exactly zero
    for zero biases).
"""

import os
import sys
import types

sys.path.insert(0, "/opt/trn_rl_repo")

import numpy as np
import ml_dtypes

import concourse.bass as bass
import concourse.bacc as bacc
import concourse.mybir as mybir
import concourse.tile as tile
from concourse.bass_utils import run_bass_kernel_spmd

BF16 = ml_dtypes.bfloat16

NCORES = 8
B = 16          # nodes per block (segment padding unit)
TB = 128        # blocks per tile
SLOTS = 32      # output slots per tile (per stream)
FD = 1024       # psum chunk columns
SUPER = 8192    # DMA super-chunk columns
BANK_TILES = 8  # tiles per [64, 512] psum slot bank

# Stash of the last run's BassKernelResults for the dev harness.
LAST_RESULT = None


# ----------------------------------------------------------------------------
# Host-side layout
# ----------------------------------------------------------------------------

def _pack_stream(starts, cnts, segids):
    """Pack segments (start, count, global id) of one stream into blocks,
    tiles and slots.

    Returns dict with block arrays (src start, real count, local slot id)
    and slot->seg map entries [(tile, slot, seg), ...].
    """
    blk_src = []      # node index of block start
    blk_cnt = []      # real nodes in block (1..16)
    blk_sid = []      # local slot id in its tile, -1 for pad blocks
    slot_entries = []  # (tile_idx, slot_idx, global_seg)

    nb = 0            # blocks emitted
    slots_used = 0    # slots used in current tile

    def cur_tile():
        return nb // TB

    def pos_in_tile():
        return nb % TB

    def pad_to_tile_end():
        nonlocal nb, slots_used
        while nb % TB != 0:
            blk_src.append(-1)
            blk_cnt.append(0)
            blk_sid.append(-1)
            nb += 1
        slots_used = 0

    for s0, cnt, gseg in zip(starts, cnts, segids):
        if cnt == 0:
            continue
        bn = -(-cnt // B)  # ceil
        emitted = 0
        while emitted < bn:
            if pos_in_tile() == 0:
                slots_used = 0
            if slots_used >= SLOTS:
                # no slot left in this tile for a new seg-run: pad it out
                pad_to_tile_end()
            # open a slot in the current tile for this segment
            slot = slots_used
            slots_used += 1
            slot_entries.append((cur_tile(), slot, gseg))
            # emit as many blocks as fit in this tile
            room = TB - pos_in_tile()
            take = min(room, bn - emitted)
            for j in range(emitted, emitted + take):
                s = s0 + j * B
                blk_src.append(s)
                blk_cnt.append(min(B, cnt - j * B))
                blk_sid.append(slot)
                nb += 1
            emitted += take
            # if segment continues, we are at a tile boundary; loop continues
    pad_to_tile_end()
    return dict(
        blk_src=np.asarray(blk_src, np.int64),
        blk_cnt=np.asarray(blk_cnt, np.int64),
        blk_sid=np.asarray(blk_sid, np.int64),
        slot_entries=slot_entries,
    )


def _node_src_for_cols(blk_src, blk_cnt, C):
    """node source index per column (-1 = pad) for the fold layout:
    col j (within chunk q of FD columns) holds node k=(j%FD)//64 of block
    q*64 + (j%64)."""
    j = np.arange(C, dtype=np.int64)
    b = (j // FD) * (FD // B) + (j % (FD // B))
    k = (j % FD) // (FD // B)
    src = blk_src[b] + k
    src = np.where((blk_src[b] >= 0) & (k < blk_cnt[b]), src, -1)
    return src


def _gather_T(a, src):
    """a[src].T with src == -1 rows zeroed; returns [a.shape[1], len(src)]
    as bf16."""
    g = a[np.clip(src, 0, a.shape[0] - 1)]
    g[src < 0] = 0
    return np.ascontiguousarray(g.T)


def _build_core_inputs(x, h_node, ptr, seg_lo, seg_hi, C):
    """Build per-core device arrays. C is the common padded column count
    (multiple of SUPER). Returns (xcat [80,C] bf16, sids [128, 2*NT] bf16,
    slot_seg [NT*64] int32, pad_nodes_per_slot [NT*64] int32)."""
    cnts = np.diff(ptr)
    # choose stream split at a segment boundary balancing node counts
    c_cnts = cnts[seg_lo:seg_hi]
    cum = np.concatenate([[0], np.cumsum(c_cnts)])
    total = cum[-1]
    s_split = int(np.searchsorted(cum, total // 2))
    s_split = min(max(s_split, 1), seg_hi - seg_lo - 1)

    NT = C // (TB * B)
    sids = np.full((128, 2 * NT), -1.0, np.float32)
    slot_seg = np.full(NT * 2 * SLOTS, -1, np.int64)
    pad_nodes = np.zeros(NT * 2 * SLOTS, np.int64)
    xcat = np.zeros((80, C), BF16)

    for st in range(2):
        lo = seg_lo if st == 0 else seg_lo + s_split
        hi = seg_lo + s_split if st == 0 else seg_hi
        segids = np.arange(lo, hi)
        starts = ptr[lo:hi].astype(np.int64)
        pk = _pack_stream(starts, cnts[lo:hi].astype(np.int64), segids)
        nb = len(pk["blk_src"])
        assert nb <= C // B, (nb, C // B)
        blk_src = np.full(C // B, -1, np.int64)
        blk_cnt = np.zeros(C // B, np.int64)
        blk_src[:nb] = pk["blk_src"]
        blk_cnt[:nb] = pk["blk_cnt"]
        # sid table: sids[p, 2t+st] = local slot of block t*TB+p
        sid_full = np.full(C // B, -1, np.int64)
        sid_full[:nb] = pk["blk_sid"]
        sids[:, st::2] = sid_full.reshape(-1, TB).T.astype(np.float32)
        # slot -> seg map and pad-node counts (vslot = t*64 + st*32 + slot)
        for (t, slot, gseg) in pk["slot_entries"]:
            v = t * 64 + st * 32 + slot
            assert slot_seg[v] == -1
            slot_seg[v] = gseg
        real = pk["blk_cnt"]
        sidv = pk["blk_sid"]
        tidx = np.arange(nb) // TB
        padn = np.where(sidv >= 0, B - real, 0)
        vv = tidx * 64 + st * 32 + np.maximum(sidv, 0)
        np.add.at(pad_nodes, vv[sidv >= 0], padn[sidv >= 0])

        src = _node_src_for_cols(blk_src, blk_cnt, C)
        r0 = 40 * st
        xcat[r0:r0 + 8, :] = _gather_T(x, src)
        xcat[r0 + 8:r0 + 40, :] = _gather_T(h_node, src)

    return xcat, sids, slot_seg, pad_nodes


# ----------------------------------------------------------------------------
# Device program
# ----------------------------------------------------------------------------

def _build_device_program(C):
    """Trace the Bass program for per-stream column count C (multiple of
    SUPER). Returns finalized nc."""
    dt = mybir.dt
    NT = C // (TB * B)
    NSLOT = NT * 64
    NBANKS = -(-NT // BANK_TILES)
    NSLOT_PAD = NBANKS * BANK_TILES * 64

    nc = bacc.Bacc(None, target_bir_lowering=False)

    xcat = nc.dram_tensor("xcat", [80, C], dt.bfloat16, kind="ExternalInput")
    sids = nc.dram_tensor("sids", [128, 2 * NT], dt.float32, kind="ExternalInput")
    w1 = nc.dram_tensor("w1blk", [80, 128], dt.bfloat16, kind="ExternalInput")
    w2 = nc.dram_tensor("w2blk", [128, 128], dt.bfloat16, kind="ExternalInput")
    w3 = nc.dram_tensor("w3", [64, 32], dt.float32, kind="ExternalInput")
    b1s = nc.dram_tensor("b1s", [128, 1], dt.float32, kind="ExternalInput")
    b2s = nc.dram_tensor("b2s", [128, 1], dt.float32, kind="ExternalInput")
    iota = nc.dram_tensor("iota32", [128, 32], dt.bfloat16, kind="ExternalInput")
    ident = nc.dram_tensor("ident", [128, 128], dt.bfloat16, kind="ExternalInput")
    outT = nc.dram_tensor("outT", [32, NSLOT_PAD], dt.float32, kind="ExternalOutput")

    AL = mybir.AluOpType
    ACTF = mybir.ActivationFunctionType

    from contextlib import ExitStack

    with tile.TileContext(nc) as tc, ExitStack() as ctx:
        consts = ctx.enter_context(tc.tile_pool(name="consts", bufs=1))
        xin_pool = ctx.enter_context(tc.tile_pool(name="xin", bufs=3))
        h_pool = ctx.enter_context(tc.tile_pool(name="h", bufs=3))
        fold_pool = ctx.enter_context(tc.tile_pool(name="fold", bufs=2))
        pt_pool = ctx.enter_context(tc.tile_pool(name="pt", bufs=3))
        psum_mm = ctx.enter_context(tc.tile_pool(name="psum_mm", bufs=2, space="PSUM"))

        w1t = consts.tile([80, 128], dt.bfloat16)
        nc.sync.dma_start(w1t[:], w1[:])
        w2t = consts.tile([128, 128], dt.bfloat16)
        nc.sync.dma_start(w2t[:], w2[:])
        w3t = consts.tile([64, 32], dt.float32)
        nc.sync.dma_start(w3t[:], w3[:])
        b1t = consts.tile([128, 1], dt.float32)
        nc.sync.dma_start(b1t[:], b1s[:])
        b2t = consts.tile([128, 1], dt.float32)
        nc.sync.dma_start(b2t[:], b2s[:])
        iott = consts.tile([128, 32], dt.bfloat16)
        nc.sync.dma_start(iott[:], iota[:])
        idt = consts.tile([128, 128], dt.bfloat16)
        nc.sync.dma_start(idt[:], ident[:])
        sidt = consts.tile([128, 2 * NT], dt.float32)
        nc.sync.dma_start(sidt[:], sids[:])

        segsum = consts.tile([64, NSLOT_PAD], dt.float32)
        outsb = consts.tile([32, NSLOT_PAD], dt.float32)
        bp_pool = ctx.enter_context(tc.tile_pool(name="bp", bufs=3))
        nc.gpsimd.memset(segsum[:], 0.0)

        TPS = SUPER // (TB * B)  # tiles per super-chunk (4)
        PB = FD // B             # partial columns per fold group (64)

        def emit_bank(g, bank_part):
            """Transpose 8 tiles' partials, slot matmuls, evac to segsum."""
            for ti in range(BANK_TILES):
                t = g * BANK_TILES + ti
                if t >= NT:
                    continue
                pt = psum_mm.tile([128, 128], dt.bfloat16, tag="p1")
                nc.tensor.transpose(pt[:], bank_part[:, ti * TB:(ti + 1) * TB],
                                    idt[:])
                ptt = pt_pool.tile([128, 128], dt.bfloat16, tag="ptt")
                nc.scalar.copy(ptt[:], pt[:])
                sl = psum_mm.tile([64, 64], dt.float32, tag="p2")
                for st in range(2):
                    S = pt_pool.tile([128, SLOTS], dt.bfloat16, tag="S")
                    nc.gpsimd.tensor_scalar(
                        S[:], iott[:], sidt[:, 2 * t + st:2 * t + st + 1], None,
                        AL.is_equal)
                    nc.tensor.matmul(sl[:, st * SLOTS:(st + 1) * SLOTS],
                                     ptt[:, st * 64:(st + 1) * 64], S[:],
                                     start=True, stop=True)
                nc.vector.tensor_copy(segsum[:, t * 64:(t + 1) * 64], sl[:])

        # ---- phase 1: MLP + 16:1 block sums, phase 2 interleaved ----------
        G = SUPER // FD  # fold groups per super-chunk
        bank_parts = {}
        for sc in range(C // SUPER):
            xt = xin_pool.tile([80, SUPER], dt.bfloat16, tag="xt")
            nc.sync.dma_start(xt[:], xcat[:, sc * SUPER:(sc + 1) * SUPER])
            h2big = h_pool.tile([128, SUPER], dt.bfloat16, tag="h2")
            for cq in range(G):
                q = sc * G + cq
                p1 = psum_mm.tile([128, FD], dt.float32, tag="p1")
                nc.tensor.matmul(p1[:, 0:512], w1t[:],
                                 xt[:, cq * FD:cq * FD + 512],
                                 start=True, stop=True)
                nc.tensor.matmul(p1[:, 512:1024], w1t[:],
                                 xt[:, cq * FD + 512:(cq + 1) * FD],
                                 start=True, stop=True)
                h1 = h_pool.tile([128, FD], dt.bfloat16, tag="h1")
                if q % 8 == 7:
                    nc.scalar.activation(h1[:], p1[:], ACTF.Relu, bias=b1t[:],
                                         scale=1.0)
                else:
                    nc.vector.tensor_scalar(h1[:], p1[:], b1t[:], 0.0, AL.add,
                                            AL.max)
                p2 = psum_mm.tile([128, FD], dt.float32, tag="p2")
                nc.tensor.matmul(p2[:, 0:512], w2t[:], h1[:, 0:512],
                                 start=True, stop=True)
                nc.tensor.matmul(p2[:, 512:1024], w2t[:], h1[:, 512:1024],
                                 start=True, stop=True)
                nc.scalar.activation(h2big[:, cq * FD:(cq + 1) * FD], p2[:],
                                     ACTF.Relu, bias=b2t[:], scale=1.0)
            # batched fold tree over the whole super-chunk (3D APs, halving
            # within each FD-column group)
            g = sc // 2
            if sc % 2 == 0:
                bank_parts[g] = bp_pool.tile([128, 2 * TPS * TB], dt.bfloat16,
                                             tag="bp", name=f"bp_{g}")
            h2v = h2big[:].rearrange("p (g c) -> p g c", c=FD)
            f1 = fold_pool.tile([128, G, FD // 2], dt.bfloat16, tag="f1")
            nc.gpsimd.tensor_tensor(f1[:], h2v[:, :, :FD // 2],
                                    h2v[:, :, FD // 2:], op=AL.add)
            f2 = fold_pool.tile([128, G, FD // 4], dt.bfloat16, tag="f2")
            nc.vector.tensor_tensor(f2[:], f1[:, :, :FD // 4],
                                    f1[:, :, FD // 4:], op=AL.add)
            f3 = fold_pool.tile([128, G, FD // 8], dt.bfloat16, tag="f3")
            nc.vector.tensor_tensor(f3[:], f2[:, :, :FD // 8],
                                    f2[:, :, FD // 8:], op=AL.add)
            half = sc % 2
            pv = bank_parts[g][:, half * TPS * TB:(half + 1) * TPS * TB].rearrange(
                "p (g c) -> p g c", c=PB)
            nc.vector.tensor_tensor(pv[:], f3[:, :, :FD // 16],
                                    f3[:, :, FD // 16:], op=AL.add)
            if sc % 2 == 1 or sc == C // SUPER - 1:
                emit_bank(g, bank_parts.pop(g)[:])

        # ---- phase 3: final W3 matmul -------------------------------------
        for fc in range(NSLOT_PAD // 512):
            fp = psum_mm.tile([32, 512], dt.float32, tag="p2")
            nc.tensor.matmul(fp[:], w3t[:], segsum[:, fc * 512:(fc + 1) * 512],
                             start=True, stop=True)
            nc.vector.tensor_copy(outsb[:, fc * 512:(fc + 1) * 512], fp[:])
        nc.sync.dma_start(outT[:], outsb[:])

    nc.finalize()
    return nc


# ----------------------------------------------------------------------------
# Entry point
# ----------------------------------------------------------------------------

def _maybe_install_ntff_hook():
    try:
        import antenv.axon_hooks  # noqa: F401
        return
    except ImportError:
        pass
    try:
        from trn_agent_boot.trn_boot import _ntff_profile_via_ctypes
        hook = _ntff_profile_via_ctypes("/opt/axon/libaxon_pjrt.so")
        mod = types.ModuleType("antenv.axon_hooks")
        mod.get_axon_ntff_profile_hook = lambda: hook
        mod.set_axon_ntff_profile_hook = lambda h: None
        sys.modules["antenv.axon_hooks"] = mod
    except Exception:
        pass


def kernel(x, h_node, W1, b1, W2, b2, W3, b3, ptr):
    global LAST_RESULT
    x = np.asarray(x, np.float32)
    h_node = np.asarray(h_node, np.float32)
    W1 = np.asarray(W1, np.float32)
    W2 = np.asarray(W2, np.float32)
    W3 = np.asarray(W3, np.float32)
    b1 = np.asarray(b1, np.float32)
    b2 = np.asarray(b2, np.float32)
    b3 = np.asarray(b3, np.float32)
    ptr = np.asarray(ptr)
    N, F = x.shape
    E = h_node.shape[1]
    H = W1.shape[1]
    M = ptr.shape[0] - 1
    assert M % NCORES == 0
    SPC = M // NCORES  # segments per core

    cnts = np.diff(ptr.astype(np.int64))

    # per-core column counts -> common C
    core_meta = []
    cmax = 0
    for c in range(NCORES):
        lo, hi = c * SPC, (c + 1) * SPC
        c_cnts = cnts[lo:hi]
        cum = np.concatenate([[0], np.cumsum(c_cnts)])
        s_split = int(np.searchsorted(cum, cum[-1] // 2))
        s_split = min(max(s_split, 1), SPC - 1)
        for st in range(2):
            sl = c_cnts[:s_split] if st == 0 else c_cnts[s_split:]
            nb = int(np.sum(-(-sl // B)))
            # upper bound on extra blocks from tile padding: one pad-run per
            # tile-ish; just compute exactly by packing later. Use a safe
            # bound now: nb + segs (each seg can waste < 1 block) is wrong;
            # instead count via the packer below only once C is known.
            cmax = max(cmax, nb)
    # add headroom for tile-boundary padding (<= TB per cut; cuts are rare;
    # tile-end alignment costs < TB blocks per tile in the worst case only
    # when slots overflow). Use exact packing to determine the real max.
    def exact_blocks(c, st):
        lo, hi = c * SPC, (c + 1) * SPC
        c_cnts = cnts[lo:hi]
        cum = np.concatenate([[0], np.cumsum(c_cnts)])
        s_split = int(np.searchsorted(cum, cum[-1] // 2))
        s_split = min(max(s_split, 1), SPC - 1)
        l2 = lo if st == 0 else lo + s_split
        h2_ = lo + s_split if st == 0 else hi
        pk = _pack_stream(ptr.astype(np.int64)[l2:h2_],
                          cnts[l2:h2_].astype(np.int64),
                          np.arange(l2, h2_))
        return len(pk["blk_src"])

    nb_max = 0
    for c in range(NCORES):
        for st in range(2):
            nb_max = max(nb_max, exact_blocks(c, st))
    C = -(-nb_max * B // SUPER) * SUPER

    # device weight/constant tensors
    w1blk = np.zeros((80, 128), np.float32)
    w1blk[0:40, 0:64] = W1
    w1blk[40:80, 64:128] = W1
    w2blk = np.zeros((128, 128), np.float32)
    w2blk[0:64, 0:64] = W2
    w2blk[64:128, 64:128] = W2
    b1st = np.concatenate([b1, b1]).reshape(128, 1).astype(np.float32)
    b2st = np.concatenate([b2, b2]).reshape(128, 1).astype(np.float32)
    iota32 = np.broadcast_to(np.arange(SLOTS, dtype=np.float32), (128, SLOTS))
    ident = np.eye(128, dtype=np.float32)

    in_maps = []
    slot_maps = []
    for c in range(NCORES):
        xcat, sids_c, slot_seg, pad_nodes = _build_core_inputs(
            x, h_node, ptr.astype(np.int64), c * SPC, (c + 1) * SPC, C)
        in_maps.append({
            "xcat": xcat,
            "sids": sids_c,
            "w1blk": w1blk.astype(BF16),
            "w2blk": w2blk.astype(BF16),
            "w3": W3,
            "b1s": b1st,
            "b2s": b2st,
            "iota32": np.ascontiguousarray(iota32).astype(BF16),
            "ident": ident.astype(BF16),
        })
        slot_maps.append((slot_seg, pad_nodes))

    nc = _build_device_program(C)
    _maybe_install_ntff_hook()
    res = run_bass_kernel_spmd(nc, in_maps, core_ids=list(range(NCORES)))
    LAST_RESULT = res

    # host assembly
    out = np.zeros((M, E), np.float32)
    # pad-node bias correction: each pad node inside a real block contributed
    # relu(relu(b1) @ W2 + b2) to its slot's h2 sum (then @ W3 on device).
    h2c = np.maximum(np.maximum(b1, 0.0) @ W2 + b2, 0.0)
    corr = (h2c @ W3).astype(np.float32)  # [E]
    for c in range(NCORES):
        virt = res.results[c]["outT"]  # [32, NSLOT_PAD]
        slot_seg, pad_nodes = slot_maps[c]
        valid = slot_seg >= 0
        nv = slot_seg.shape[0]
        vt = virt[:, :nv].T  # [NSLOT, 32]
        np.add.at(out, slot_seg[valid], vt[valid])
        np.add.at(out, slot_seg[valid],
                  -pad_nodes[valid, None].astype(np.float32) * corr[None, :])
    out += cnts[:, None].astype(np.float32) * b3[None, :]
    return out



# revision 3
# speedup vs baseline: 2.4078x; 2.4078x over previous
"""DagEncoder (MLP + segment_sum) Trainium2 kernel, 8-core SPMD.

Contract: kernel(**inputs) takes the FULL unsharded inputs of
reference.setup_inputs() and returns the FULL [M, E] output.

Strategy (pure data parallelism over DAG segments):
  - 20000 segments split into 8 cores x 2500 segments; each core's segments
    are split into 2 "streams" at a node-count midpoint so two nodes are
    processed per PE column (feature-major layout, 2x40 features stacked on
    partitions 0..79).
  - Nodes are grouped into blocks of B=8 (per segment, zero-padded to a
    multiple of 8). Blocks are packed into regions of W columns; node s of
    block b lives at column b of sub-chunk s, so the 8:1 block reduction is
    expressed as 8 successive ops on the SAME [128, W] accumulator:
      s=0: ACT  acc_a = relu(p2 + b2)            (activation, exact bias)
      s=1: DVE  acc_b = relu(p2 + b2)            (tensor_scalar)
      s>=2: DVE acc_{a/b} = max(p2,0) + acc      (scalar_tensor_tensor;
                                                  requires b2 == 0, which
                                                  holds for this problem;
                                                  a 2-instr fallback covers
                                                  the general case)
  - Per 1024-col chunk: mm1 (W1 blockdiag, 80->128) -> ACT relu evac h1
    (exact b1) -> mm2 (W2 blockdiag) -> h2 consume as above. GpSimd combines
    acc_a + acc_b per region; block partial sums are DMA'd out.
  - Host: per-segment sums of block partials (cumsum-diff; blocks of one
    segment are consecutive), then @W3 + counts*b3 (linear ops commute with
    the segment sum), plus a pad-slot correction (zero for zero biases).
"""

import sys
import types

sys.path.insert(0, "/opt/trn_rl_repo")

import numpy as np
import ml_dtypes

import concourse.bass as bass  # noqa: F401  (side-effect imports)
import concourse.bacc as bacc
import concourse.mybir as mybir
import concourse.tile as tile
from concourse.bass_utils import run_bass_kernel_spmd

BF16 = ml_dtypes.bfloat16

NCORES = 8
B = 8            # nodes per block (segment padding unit, = fold depth)
W_FULL = 1024    # blocks per full region (acc width)
W_GRAN = 256     # tail-region width granularity
CHUNK = 1024     # psum chunk columns

# Stash of the last run's BassKernelResults for the dev harness.
LAST_RESULT = None


# ----------------------------------------------------------------------------
# Host-side layout
# ----------------------------------------------------------------------------

def _stream_split(cnts_c):
    """Split a core's segments into 2 streams at a node-count midpoint."""
    cum = np.concatenate([[0], np.cumsum(cnts_c)])
    s_split = int(np.searchsorted(cum, cum[-1] // 2))
    return min(max(s_split, 1), len(cnts_c) - 1)


def _stream_blocks(starts, cnts):
    """Block arrays for one stream: (blk_src, blk_cnt, nb_per_seg)."""
    nb_per_seg = -(-cnts // B)          # ceil; 0 for empty segments
    nblocks = int(nb_per_seg.sum())
    seg_of_blk = np.repeat(np.arange(len(cnts)), nb_per_seg)
    blk_starts = np.concatenate([[0], np.cumsum(nb_per_seg)])
    within = np.arange(nblocks) - blk_starts[seg_of_blk]
    blk_src = np.repeat(starts, nb_per_seg) + B * within
    blk_cnt = np.minimum(B, np.repeat(cnts, nb_per_seg) - B * within)
    return blk_src, blk_cnt, nb_per_seg


def _region_plan(blocks_padded):
    """List of (blk_off, W) regions covering blocks_padded block columns."""
    plan = []
    off = 0
    while off < blocks_padded:
        w = min(W_FULL, blocks_padded - off)
        plan.append((off, w))
        off += w
    return plan


def _col_src(blk_src, blk_cnt, plan):
    """node source index per xcat column (-1 = zero pad) for the layout:
    region r, sub-chunk s (0..B-1), col c -> node s of block blk_off+c."""
    total_blocks = sum(w for _, w in plan)
    out = np.empty(total_blocks * B, np.int64)
    col = 0
    for blk_off, w in plan:
        bs = blk_src[blk_off:blk_off + w]
        bc = blk_cnt[blk_off:blk_off + w]
        s = np.arange(B)[:, None]
        srcs = bs[None, :] + s
        valid = (bs[None, :] >= 0) & (s < bc[None, :])
        out[col:col + B * w] = np.where(valid, srcs, -1).reshape(-1)
        col += B * w
    return out


def _gather_T(a, src):
    """a[src].T with src == -1 rows zeroed; returns [a.shape[1], len(src)]
    as bf16."""
    g = a[np.clip(src, 0, a.shape[0] - 1)]
    g[src < 0] = 0
    return np.ascontiguousarray(g.T.astype(BF16))


# ----------------------------------------------------------------------------
# Device program
# ----------------------------------------------------------------------------

def _build_device_program(plan, zero_b2):
    """Trace the Bass program for the given region plan."""
    dt = mybir.dt
    AL = mybir.AluOpType
    ACTF = mybir.ActivationFunctionType

    TOTB = sum(w for _, w in plan)   # total block columns
    C = TOTB * B                     # xcat columns

    nc = bacc.Bacc(None, target_bir_lowering=False)

    xcat = nc.dram_tensor("xcat", [80, C], dt.bfloat16, kind="ExternalInput")
    w1 = nc.dram_tensor("w1blk", [80, 128], dt.bfloat16, kind="ExternalInput")
    w2 = nc.dram_tensor("w2blk", [128, 128], dt.bfloat16, kind="ExternalInput")
    b1s = nc.dram_tensor("b1s", [128, 1], dt.float32, kind="ExternalInput")
    b2s = nc.dram_tensor("b2s", [128, 1], dt.float32, kind="ExternalInput")
    outT = nc.dram_tensor("outT", [128, TOTB], dt.float32, kind="ExternalOutput")

    from contextlib import ExitStack

    with tile.TileContext(nc) as tc, ExitStack() as ctx:
        consts = ctx.enter_context(tc.tile_pool(name="consts", bufs=1))
        xin_pool = ctx.enter_context(tc.tile_pool(name="xin", bufs=3))
        h1_pool = ctx.enter_context(tc.tile_pool(name="h1p", bufs=3))
        acc_pool = ctx.enter_context(tc.tile_pool(name="accp", bufs=2))
        out_pool = ctx.enter_context(tc.tile_pool(name="outp", bufs=2))
        psum = ctx.enter_context(tc.tile_pool(name="psum", bufs=2, space="PSUM"))

        w1t = consts.tile([80, 128], dt.bfloat16)
        nc.sync.dma_start(w1t[:], w1[:])
        w2t = consts.tile([128, 128], dt.bfloat16)
        nc.sync.dma_start(w2t[:], w2[:])
        b1t = consts.tile([128, 1], dt.float32)
        nc.sync.dma_start(b1t[:], b1s[:])
        b2t = consts.tile([128, 1], dt.float32)
        nc.sync.dma_start(b2t[:], b2s[:])

        NR = len(plan)

        # chunk list: (region, s, col_off, width)
        chunks = []
        col = 0
        for r, (blk_off, w) in enumerate(plan):
            for s in range(B):
                chunks.append((r, s, col, w))
                col += w
        NQ = len(chunks)

        xts = {}       # region -> xcat SBUF tile
        accs = {}      # region -> (acc_a, acc_b)
        p1s = {}       # q -> p1 psum tile (pending mm1 -> h1 evac)
        p2s = {}       # q -> p2 psum tile (pending h2 consume)
        h1s = {}       # q -> h1 SBUF tile

        def dma_in(r):
            blk_off, w = plan[r]
            xt = xin_pool.tile([80, B * w], dt.bfloat16, tag="xt",
                               name=f"xt_{r}")
            nc.sync.dma_start(xt[:], xcat[:, blk_off * B:(blk_off + w) * B])
            xts[r] = xt

        def emit_mm1(q):
            r, s, co, w = chunks[q]
            p1 = psum.tile([128, w], dt.float32, tag="p1", name=f"p1_{q}")
            xt = xts[r]
            xo = (co - plan[r][0] * B)
            for o in range(0, w, 512):
                n = min(512, w - o)
                nc.tensor.matmul(p1[:, o:o + n], w1t[:],
                                 xt[:, xo + o:xo + o + n],
                                 start=True, stop=True)
            p1s[q] = p1

        def emit_h1_mm2(q):
            r, s, co, w = chunks[q]
            p1 = p1s.pop(q)
            h1 = h1_pool.tile([128, w], dt.bfloat16, tag="h1", name=f"h1_{q}")
            nc.scalar.activation(h1[:], p1[:], ACTF.Relu, bias=b1t[:],
                                 scale=1.0)
            p2 = psum.tile([128, w], dt.float32, tag="p2", name=f"p2_{q}")
            for o in range(0, w, 512):
                n = min(512, w - o)
                nc.tensor.matmul(p2[:, o:o + n], w2t[:], h1[:, o:o + n],
                                 start=True, stop=True)
            p2s[q] = p2
            h1s[q] = h1

        def emit_h2(q):
            r, s, co, w = chunks[q]
            p2 = p2s.pop(q)
            h1s.pop(q, None)
            if s == 0:
                acc_a = acc_pool.tile([128, w], dt.bfloat16, tag="acca",
                                      name=f"acca_{r}")
                acc_b = acc_pool.tile([128, w], dt.bfloat16, tag="accb",
                                      name=f"accb_{r}")
                accs[r] = (acc_a, acc_b)
                nc.scalar.activation(acc_a[:], p2[:], ACTF.Relu, bias=b2t[:],
                                     scale=1.0)
                return
            acc_a, acc_b = accs[r]
            acc = acc_a if s % 2 == 0 else acc_b
            if s == 1:
                nc.vector.tensor_scalar(acc[:], p2[:], b2t[:], 0.0,
                                        AL.add, AL.max)
            elif zero_b2:
                nc.vector.scalar_tensor_tensor(acc[:], p2[:], 0.0, acc[:],
                                               AL.max, AL.add)
            else:
                tmp = h1_pool.tile([128, w], dt.bfloat16, tag="tmp",
                                   name=f"tmp_{q}")
                nc.vector.tensor_scalar(tmp[:], p2[:], b2t[:], 0.0,
                                        AL.add, AL.max)
                nc.vector.tensor_tensor(acc[:], tmp[:], acc[:], op=AL.add)
            if s == B - 1:
                blk_off, _ = plan[r]
                oc = out_pool.tile([128, w], dt.float32, tag="oc",
                                   name=f"oc_{r}")
                nc.gpsimd.tensor_tensor(oc[:], acc_a[:], acc_b[:], op=AL.add)
                nc.sync.dma_start(outT[:, blk_off:blk_off + w], oc[:])
                accs.pop(r)

        # prologue: prefetch first 3 regions, fill mm1 for chunk 0
        for r in range(min(3, NR)):
            dma_in(r)
        emit_mm1(0)
        # steady state: iteration q emits mm1(q+1), h1+mm2(q), h2(q-1)
        for q in range(NQ):
            r, s, co, w = chunks[q]
            # prefetch region r+3 once every mm1 read of region r is emitted
            # (its xin buffer slot is the one being recycled)
            if s == B - 1 and r + 3 < NR:
                dma_in(r + 3)
            if q + 1 < NQ:
                emit_mm1(q + 1)
            emit_h1_mm2(q)
            if q > 0:
                emit_h2(q - 1)
        emit_h2(NQ - 1)

    nc.finalize()
    return nc


# ----------------------------------------------------------------------------
# Entry point
# ----------------------------------------------------------------------------

def _maybe_install_ntff_hook():
    try:
        import antenv.axon_hooks  # noqa: F401
        return
    except ImportError:
        pass
    try:
        from trn_agent_boot.trn_boot import _ntff_profile_via_ctypes
        hook = _ntff_profile_via_ctypes("/opt/axon/libaxon_pjrt.so")
        mod = types.ModuleType("antenv.axon_hooks")
        mod.get_axon_ntff_profile_hook = lambda: hook
        mod.set_axon_ntff_profile_hook = lambda h: None
        sys.modules["antenv.axon_hooks"] = mod
    except Exception:
        pass


def kernel(x, h_node, W1, b1, W2, b2, W3, b3, ptr):
    global LAST_RESULT
    x = np.asarray(x, np.float32)
    h_node = np.asarray(h_node, np.float32)
    W1 = np.asarray(W1, np.float32)
    W2 = np.asarray(W2, np.float32)
    W3 = np.asarray(W3, np.float32)
    b1 = np.asarray(b1, np.float32)
    b2 = np.asarray(b2, np.float32)
    b3 = np.asarray(b3, np.float32)
    ptr = np.asarray(ptr).astype(np.int64)
    N, F = x.shape
    E = h_node.shape[1]
    M = ptr.shape[0] - 1
    assert M % NCORES == 0
    SPC = M // NCORES

    cnts = np.diff(ptr)

    # per-core/stream block arrays and the common padded block count
    core_streams = []
    blk_max = 0
    for c in range(NCORES):
        lo, hi = c * SPC, (c + 1) * SPC
        sp = _stream_split(cnts[lo:hi])
        streams = []
        for st in range(2):
            l2 = lo if st == 0 else lo + sp
            h2 = lo + sp if st == 0 else hi
            blk_src, blk_cnt, nb_per_seg = _stream_blocks(
                ptr[l2:h2], cnts[l2:h2])
            streams.append((l2, h2, blk_src, blk_cnt, nb_per_seg))
            blk_max = max(blk_max, len(blk_src))
        core_streams.append(streams)

    blocks_padded = -(-blk_max // W_GRAN) * W_GRAN
    plan = _region_plan(blocks_padded)
    TOTB = blocks_padded
    C = TOTB * B

    # device weight/constant tensors
    w1blk = np.zeros((80, 128), np.float32)
    w1blk[0:40, 0:64] = W1
    w1blk[40:80, 64:128] = W1
    w2blk = np.zeros((128, 128), np.float32)
    w2blk[0:64, 0:64] = W2
    w2blk[64:128, 64:128] = W2
    b1st = np.concatenate([b1, b1]).reshape(128, 1).astype(np.float32)
    b2st = np.concatenate([b2, b2]).reshape(128, 1).astype(np.float32)

    in_maps = []
    for c in range(NCORES):
        xcat = np.zeros((80, C), BF16)
        for st, (l2, h2, blk_src, blk_cnt, nb_per_seg) in \
                enumerate(core_streams[c]):
            bs = np.full(TOTB, -1, np.int64)
            bc = np.zeros(TOTB, np.int64)
            bs[:len(blk_src)] = blk_src
            bc[:len(blk_cnt)] = blk_cnt
            src = _col_src(bs, bc, plan)
            r0 = 40 * st
            xcat[r0:r0 + 8, :] = _gather_T(x, src)
            xcat[r0 + 8:r0 + 40, :] = _gather_T(h_node, src)
        in_maps.append({
            "xcat": xcat,
            "w1blk": w1blk.astype(BF16),
            "w2blk": w2blk.astype(BF16),
            "b1s": b1st,
            "b2s": b2st,
        })

    zero_b2 = bool(np.all(b2 == 0.0))
    nc = _build_device_program(plan, zero_b2)
    _maybe_install_ntff_hook()
    res = run_bass_kernel_spmd(nc, in_maps, core_ids=list(range(NCORES)))
    LAST_RESULT = res

    # host assembly: block partials -> segment sums -> @W3 + bias terms
    out = np.zeros((M, E), np.float32)
    # each empty slot inside a real block contributed relu(relu(b1)@W2 + b2)
    h2c = np.maximum(np.maximum(b1, 0.0) @ W2 + b2, 0.0)
    corr = (h2c @ W3).astype(np.float32)  # [E]
    for c in range(NCORES):
        P = np.asarray(res.results[c]["outT"], np.float32)  # [128, TOTB]
        for st, (l2, h2, blk_src, blk_cnt, nb_per_seg) in \
                enumerate(core_streams[c]):
            nb = len(blk_src)
            p_st = P[st * 64:(st + 1) * 64, :nb].T  # [nb, 64]
            cs = np.concatenate([np.zeros((1, 64), np.float64),
                                 np.cumsum(p_st, axis=0, dtype=np.float64)])
            ends = np.cumsum(nb_per_seg)
            starts = ends - nb_per_seg
            h2sum = (cs[ends] - cs[starts]).astype(np.float32)  # [nsegs, 64]
            segs = np.arange(l2, h2)
            pad_slots = (B * nb_per_seg - cnts[l2:h2]).astype(np.float32)
            out[segs] = (h2sum @ W3
                         + cnts[l2:h2, None].astype(np.float32) * b3[None, :]
                         - pad_slots[:, None] * corr[None, :])
    return out


# revision 9
# speedup vs baseline: 2.5587x; 1.0627x over previous
"""DagEncoder (MLP + segment_sum) Trainium2 kernel, 8-core SPMD.

Contract: kernel(**inputs) takes the FULL unsharded inputs of
reference.setup_inputs() and returns the FULL [M, E] output.

Strategy (pure data parallelism over DAG segments):
  - 20000 segments split into 8 cores x 2500 segments; each core's segments
    are split into 2 "streams" at a node-count midpoint so two nodes are
    processed per PE column (feature-major layout, 2x40 features stacked on
    partitions 0..79).
  - Nodes are grouped into blocks of B=8 (per segment, zero-padded to a
    multiple of 8). Blocks are packed into regions of W columns; node s of
    block b lives at column b of sub-chunk s, so the 8:1 block reduction is
    expressed as 8 successive ops on the SAME [128, W] accumulator:
      s=0: ACT  acc_a = relu(p2 + b2)            (activation, exact bias)
      s=1: DVE  acc_b = relu(p2 + b2)            (tensor_scalar)
      s>=2: DVE acc_{a/b} = max(p2,0) + acc      (scalar_tensor_tensor;
                                                  requires b2 == 0, which
                                                  holds for this problem;
                                                  a 2-instr fallback covers
                                                  the general case)
  - Per 1024-col chunk: mm1 (W1 blockdiag, 80->128) -> ACT relu evac h1
    (exact b1) -> mm2 (W2 blockdiag) -> h2 consume as above. GpSimd combines
    acc_a + acc_b per region; block partial sums are DMA'd out.
  - Host: per-segment sums of block partials (cumsum-diff; blocks of one
    segment are consecutive), then @W3 + counts*b3 (linear ops commute with
    the segment sum), plus a pad-slot correction (zero for zero biases).
"""

import sys
import types

sys.path.insert(0, "/opt/trn_rl_repo")

import numpy as np
import ml_dtypes

import concourse.bass as bass  # noqa: F401  (side-effect imports)
import concourse.bacc as bacc
import concourse.mybir as mybir
import concourse.tile as tile
from concourse.bass_utils import run_bass_kernel_spmd

BF16 = ml_dtypes.bfloat16

NCORES = 8
B = 8            # nodes per block (segment padding unit, = fold depth)
W_FULL = 1024    # blocks per full region (acc width)
W_GRAN = 256     # tail-region width granularity
CHUNK = 1024     # psum chunk columns

# Stash of the last run's BassKernelResults for the dev harness.
LAST_RESULT = None


# ----------------------------------------------------------------------------
# Host-side layout
# ----------------------------------------------------------------------------

def _stream_bounds(cnts):
    """Split all segments into 2*NCORES contiguous ranges with near-equal
    block counts (core c gets streams 2c and 2c+1)."""
    nb = -(-cnts // B)
    tot = int(nb.sum())
    cum = np.concatenate([[0], np.cumsum(nb)])
    ns = 2 * NCORES
    bounds = [int(np.searchsorted(cum, round(tot * j / ns)))
              for j in range(ns + 1)]
    bounds[0], bounds[-1] = 0, len(cnts)
    return bounds


def _stream_blocks(starts, cnts):
    """Block arrays for one stream: (blk_src, blk_cnt, nb_per_seg)."""
    nb_per_seg = -(-cnts // B)          # ceil; 0 for empty segments
    nblocks = int(nb_per_seg.sum())
    seg_of_blk = np.repeat(np.arange(len(cnts)), nb_per_seg)
    blk_starts = np.concatenate([[0], np.cumsum(nb_per_seg)])
    within = np.arange(nblocks) - blk_starts[seg_of_blk]
    blk_src = np.repeat(starts, nb_per_seg) + B * within
    blk_cnt = np.minimum(B, np.repeat(cnts, nb_per_seg) - B * within)
    return blk_src, blk_cnt, nb_per_seg


def _region_plan(blocks_padded):
    """List of (blk_off, W) regions covering blocks_padded block columns."""
    plan = []
    off = 0
    while off < blocks_padded:
        w = min(W_FULL, blocks_padded - off)
        plan.append((off, w))
        off += w
    return plan


def _col_src(blk_src, blk_cnt, plan):
    """node source index per xcat column (-1 = zero pad) for the layout:
    region r, sub-chunk s (0..B-1), col c -> node s of block blk_off+c."""
    total_blocks = sum(w for _, w in plan)
    out = np.empty(total_blocks * B, np.int64)
    col = 0
    for blk_off, w in plan:
        bs = blk_src[blk_off:blk_off + w]
        bc = blk_cnt[blk_off:blk_off + w]
        s = np.arange(B)[:, None]
        srcs = bs[None, :] + s
        valid = (bs[None, :] >= 0) & (s < bc[None, :])
        out[col:col + B * w] = np.where(valid, srcs, -1).reshape(-1)
        col += B * w
    return out


def _gather_T(a, src):
    """a[src].T with src == -1 rows zeroed; returns [a.shape[1], len(src)]
    as bf16."""
    g = a[np.clip(src, 0, a.shape[0] - 1)]
    g[src < 0] = 0
    return np.ascontiguousarray(g.T.astype(BF16))


# ----------------------------------------------------------------------------
# Device program
# ----------------------------------------------------------------------------

def _build_device_program(plan, zero_b2):
    """Trace the Bass program for the given region plan."""
    dt = mybir.dt
    AL = mybir.AluOpType
    ACTF = mybir.ActivationFunctionType

    TOTB = sum(w for _, w in plan)   # total block columns
    C = TOTB * B                     # xcat columns

    nc = bacc.Bacc(None, target_bir_lowering=False)

    xcat = nc.dram_tensor("xcat", [80, C], dt.bfloat16, kind="ExternalInput")
    w1 = nc.dram_tensor("w1blk", [80, 128], dt.bfloat16, kind="ExternalInput")
    w2 = nc.dram_tensor("w2blk", [128, 128], dt.bfloat16, kind="ExternalInput")
    b1s = nc.dram_tensor("b1s", [128, 1], dt.float32, kind="ExternalInput")
    b2s = nc.dram_tensor("b2s", [128, 1], dt.float32, kind="ExternalInput")
    outT = nc.dram_tensor("outT", [128, TOTB], dt.bfloat16,
                          kind="ExternalOutput")
    w_last = plan[-1][1]
    outA = nc.dram_tensor("outA", [128, w_last], dt.bfloat16,
                          kind="ExternalOutput")
    outB = nc.dram_tensor("outB", [128, w_last], dt.bfloat16,
                          kind="ExternalOutput")

    from contextlib import ExitStack

    with tile.TileContext(nc) as tc, ExitStack() as ctx:
        consts = ctx.enter_context(tc.tile_pool(name="consts", bufs=1))
        xin_pool = ctx.enter_context(tc.tile_pool(name="xin", bufs=3))
        h1_pool = ctx.enter_context(tc.tile_pool(name="h1p", bufs=3))
        acc_pool = ctx.enter_context(tc.tile_pool(name="accp", bufs=2))
        out_pool = ctx.enter_context(tc.tile_pool(name="outp", bufs=2))
        psum = ctx.enter_context(tc.tile_pool(name="psum", bufs=2, space="PSUM"))

        w1t = consts.tile([80, 128], dt.bfloat16)
        nc.sync.dma_start(w1t[:], w1[:])
        w2t = consts.tile([128, 128], dt.bfloat16)
        nc.sync.dma_start(w2t[:], w2[:])
        b1t = consts.tile([128, 1], dt.float32)
        nc.sync.dma_start(b1t[:], b1s[:])
        b2t = consts.tile([128, 1], dt.float32)
        nc.sync.dma_start(b2t[:], b2s[:])

        NR = len(plan)

        # chunk list: (region, s, col_off, width)
        chunks = []
        col = 0
        for r, (blk_off, w) in enumerate(plan):
            for s in range(B):
                chunks.append((r, s, col, w))
                col += w
        NQ = len(chunks)

        xts = {}       # region -> xcat SBUF tile
        accs = {}      # region -> (acc_a, acc_b)
        p1s = {}       # q -> p1 psum tile (pending mm1 -> h1 evac)
        p2s = {}       # q -> p2 psum tile (pending h2 consume)
        h1s = {}       # q -> h1 SBUF tile

        def dma_in(r):
            blk_off, w = plan[r]
            cw = B * w
            xt = xin_pool.tile([80, cw], dt.bfloat16, tag="xt",
                               name=f"xt_{r}")
            # split across 4 DMA rings for parallelism + faster first chunk
            npc = 4 if cw % 4 == 0 else 1
            pw = cw // npc
            for p in range(npc):
                nc.sync.dma_start(
                    xt[:, p * pw:(p + 1) * pw],
                    xcat[:, blk_off * B + p * pw:blk_off * B + (p + 1) * pw])
            xts[r] = xt

        def emit_mm1(q):
            r, s, co, w = chunks[q]
            p1 = psum.tile([128, w], dt.float32, tag="p1", name=f"p1_{q}")
            xt = xts[r]
            xo = (co - plan[r][0] * B)
            for o in range(0, w, 512):
                n = min(512, w - o)
                nc.tensor.matmul(p1[:, o:o + n], w1t[:],
                                 xt[:, xo + o:xo + o + n],
                                 start=True, stop=True)
            p1s[q] = p1

        def emit_h1_mm2(q):
            r, s, co, w = chunks[q]
            p1 = p1s.pop(q)
            h1 = h1_pool.tile([128, w], dt.bfloat16, tag="h1", name=f"h1_{q}")
            nc.scalar.activation(h1[:], p1[:], ACTF.Relu, bias=b1t[:],
                                 scale=1.0)
            p2 = psum.tile([128, w], dt.float32, tag="p2", name=f"p2_{q}")
            for o in range(0, w, 512):
                n = min(512, w - o)
                nc.tensor.matmul(p2[:, o:o + n], w2t[:], h1[:, o:o + n],
                                 start=True, stop=True)
            p2s[q] = p2
            h1s[q] = h1

        def emit_h2(q):
            r, s, co, w = chunks[q]
            p2 = p2s.pop(q)
            h1s.pop(q, None)
            last_r = r == NR - 1
            if s == 0:
                acc_a = acc_pool.tile([128, w], dt.bfloat16, tag="acca",
                                      name=f"acca_{r}")
                acc_b = acc_pool.tile([128, w], dt.bfloat16, tag="accb",
                                      name=f"accb_{r}")
                accs[r] = (acc_a, acc_b)
                if w > 512:
                    # split at the psum bank boundary: ACT bank 0, DVE bank 1
                    nc.scalar.activation(acc_a[:, :512], p2[:, :512],
                                         ACTF.Relu, bias=b2t[:], scale=1.0)
                    nc.vector.tensor_scalar(acc_a[:, 512:], p2[:, 512:],
                                            b2t[:], 0.0, AL.add, AL.max)
                else:
                    nc.scalar.activation(acc_a[:], p2[:], ACTF.Relu,
                                         bias=b2t[:], scale=1.0)
                return
            acc_a, acc_b = accs[r]
            acc = acc_a if s % 2 == 0 else acc_b
            if s == 1:
                nc.vector.tensor_scalar(acc[:], p2[:], b2t[:], 0.0,
                                        AL.add, AL.max)
            elif zero_b2:
                nc.vector.scalar_tensor_tensor(acc[:], p2[:], 0.0, acc[:],
                                               AL.max, AL.add)
            else:
                tmp = h1_pool.tile([128, w], dt.bfloat16, tag="tmp",
                                   name=f"tmp_{q}")
                nc.vector.tensor_scalar(tmp[:], p2[:], b2t[:], 0.0,
                                        AL.add, AL.max)
                nc.vector.tensor_tensor(acc[:], tmp[:], acc[:], op=AL.add)
            if s == B - 1:
                blk_off, _ = plan[r]
                if last_r:
                    # drain fast: ship both accumulators, host adds them
                    nc.sync.dma_start(outA[:, :w], acc_a[:])
                    nc.sync.dma_start(outB[:, :w], acc_b[:])
                else:
                    oc = out_pool.tile([128, w], dt.bfloat16, tag="oc",
                                       name=f"oc_{r}")
                    nc.gpsimd.tensor_tensor(oc[:], acc_a[:], acc_b[:],
                                            op=AL.add)
                    nc.sync.dma_start(outT[:, blk_off:blk_off + w], oc[:])
                accs.pop(r)

        # prologue: prefetch first 3 regions, fill mm1 for chunk 0
        for r in range(min(3, NR)):
            dma_in(r)
        emit_mm1(0)
        # steady state: iteration q emits mm1(q+1), h1+mm2(q), h2(q-1)
        for q in range(NQ):
            r, s, co, w = chunks[q]
            # prefetch region r+3 once every mm1 read of region r is emitted
            # (its xin buffer slot is the one being recycled)
            if s == B - 1 and r + 3 < NR:
                dma_in(r + 3)
            if q + 1 < NQ:
                emit_mm1(q + 1)
            emit_h1_mm2(q)
            if q > 0:
                emit_h2(q - 1)
        emit_h2(NQ - 1)

    nc.finalize()
    return nc


# ----------------------------------------------------------------------------
# Entry point
# ----------------------------------------------------------------------------

def _maybe_install_ntff_hook():
    try:
        import antenv.axon_hooks  # noqa: F401
        return
    except ImportError:
        pass
    try:
        from trn_agent_boot.trn_boot import _ntff_profile_via_ctypes
        hook = _ntff_profile_via_ctypes("/opt/axon/libaxon_pjrt.so")
        mod = types.ModuleType("antenv.axon_hooks")
        mod.get_axon_ntff_profile_hook = lambda: hook
        mod.set_axon_ntff_profile_hook = lambda h: None
        sys.modules["antenv.axon_hooks"] = mod
    except Exception:
        pass


def kernel(x, h_node, W1, b1, W2, b2, W3, b3, ptr):
    global LAST_RESULT
    x = np.asarray(x, np.float32)
    h_node = np.asarray(h_node, np.float32)
    W1 = np.asarray(W1, np.float32)
    W2 = np.asarray(W2, np.float32)
    W3 = np.asarray(W3, np.float32)
    b1 = np.asarray(b1, np.float32)
    b2 = np.asarray(b2, np.float32)
    b3 = np.asarray(b3, np.float32)
    ptr = np.asarray(ptr).astype(np.int64)
    N, F = x.shape
    E = h_node.shape[1]
    M = ptr.shape[0] - 1

    cnts = np.diff(ptr)

    # per-core/stream block arrays and the common padded block count
    bounds = _stream_bounds(cnts)
    core_streams = []
    blk_max = 0
    for c in range(NCORES):
        streams = []
        for st in range(2):
            l2, h2 = bounds[2 * c + st], bounds[2 * c + st + 1]
            blk_src, blk_cnt, nb_per_seg = _stream_blocks(
                ptr[l2:h2], cnts[l2:h2])
            streams.append((l2, h2, blk_src, blk_cnt, nb_per_seg))
            blk_max = max(blk_max, len(blk_src))
        core_streams.append(streams)

    blocks_padded = -(-blk_max // W_GRAN) * W_GRAN
    plan = _region_plan(blocks_padded)
    TOTB = blocks_padded
    C = TOTB * B

    # device weight/constant tensors
    w1blk = np.zeros((80, 128), np.float32)
    w1blk[0:40, 0:64] = W1
    w1blk[40:80, 64:128] = W1
    w2blk = np.zeros((128, 128), np.float32)
    w2blk[0:64, 0:64] = W2
    w2blk[64:128, 64:128] = W2
    b1st = np.concatenate([b1, b1]).reshape(128, 1).astype(np.float32)
    b2st = np.concatenate([b2, b2]).reshape(128, 1).astype(np.float32)

    in_maps = []
    for c in range(NCORES):
        xcat = np.zeros((80, C), BF16)
        for st, (l2, h2, blk_src, blk_cnt, nb_per_seg) in \
                enumerate(core_streams[c]):
            bs = np.full(TOTB, -1, np.int64)
            bc = np.zeros(TOTB, np.int64)
            bs[:len(blk_src)] = blk_src
            bc[:len(blk_cnt)] = blk_cnt
            src = _col_src(bs, bc, plan)
            r0 = 40 * st
            xcat[r0:r0 + 8, :] = _gather_T(x, src)
            xcat[r0 + 8:r0 + 40, :] = _gather_T(h_node, src)
        in_maps.append({
            "xcat": xcat,
            "w1blk": w1blk.astype(BF16),
            "w2blk": w2blk.astype(BF16),
            "b1s": b1st,
            "b2s": b2st,
        })

    zero_b2 = bool(np.all(b2 == 0.0))
    nc = _build_device_program(plan, zero_b2)
    _maybe_install_ntff_hook()
    res = run_bass_kernel_spmd(nc, in_maps, core_ids=list(range(NCORES)))
    LAST_RESULT = res

    # host assembly: block partials -> segment sums -> @W3 + bias terms
    out = np.zeros((M, E), np.float32)
    # each empty slot inside a real block contributed relu(relu(b1)@W2 + b2)
    h2c = np.maximum(np.maximum(b1, 0.0) @ W2 + b2, 0.0)
    corr = (h2c @ W3).astype(np.float32)  # [E]
    last_off, w_last = plan[-1]
    for c in range(NCORES):
        P = np.asarray(res.results[c]["outT"], np.float32)  # [128, TOTB]
        P[:, last_off:last_off + w_last] = (
            np.asarray(res.results[c]["outA"], np.float32)
            + np.asarray(res.results[c]["outB"], np.float32))
        for st, (l2, h2, blk_src, blk_cnt, nb_per_seg) in \
                enumerate(core_streams[c]):
            nb = len(blk_src)
            p_st = P[st * 64:(st + 1) * 64, :nb].T  # [nb, 64]
            cs = np.concatenate([np.zeros((1, 64), np.float64),
                                 np.cumsum(p_st, axis=0, dtype=np.float64)])
            ends = np.cumsum(nb_per_seg)
            starts = ends - nb_per_seg
            h2sum = (cs[ends] - cs[starts]).astype(np.float32)  # [nsegs, 64]
            segs = np.arange(l2, h2)
            pad_slots = (B * nb_per_seg - cnts[l2:h2]).astype(np.float32)
            out[segs] = (h2sum @ W3
                         + cnts[l2:h2, None].astype(np.float32) * b3[None, :]
                         - pad_slots[:, None] * corr[None, :])
    return out
